# revision 37
# baseline (speedup 1.0000x reference)
"""DeepFM (embedding_lookup) Trainium2 Bass kernel.

Sharding: data-parallel on batch across 8 NeuronCores; the embedding
table is replicated per core in bf16, padded to 64B rows packed 4-per-
256B slot so the SWDGE dma_gather ucode (int16 slot indices, 256B-
multiple stride) can fetch each sample-feature row as one descriptor.
Gathers are spread over 4 SWDGE queues (4 Q7 cpu pairs generate
descriptors concurrently). A predicated-copy tree on the vector engine
selects the right 64B row out of each 256B slot, split by feature
group so selection overlaps the remaining gathers. FM statistics are
computed from the pre-reorder (feature-major) layout so only the
transposes wait on the X' reorder. The batch is gathered in two uneven
halves (12 chunks + 4 chunks) so most of the MLP overlaps the second
gather wave and only one block remains after the last gather. MLP runs
in bf16 with fp32 PSUM; the FM row is accumulated straight into the
output-layer PSUM via identity matmuls.

Self-contained: hardcodes all shapes from the problem spec.
"""

import numpy as np
import ml_dtypes

import concourse.bass as bass
import concourse.bacc as bacc
import concourse.mybir as mybir
import concourse.tile as tile
from concourse.bass_utils import run_bass_kernel_spmd
from concourse.masks import make_identity

F32 = mybir.dt.float32
BF16 = mybir.dt.bfloat16
I16 = mybir.dt.int16
U8 = mybir.dt.uint8
AF = mybir.ActivationFunctionType
ALU = mybir.AluOpType
BF = ml_dtypes.bfloat16

# Problem dims
B, NCONT, F, V, D = 16384, 13, 26, 100000, 16
H1, H2 = 400, 400
NCORES = 8
BC = B // NCORES          # 2048 rows per core
SUB = 128                 # batch subtile (partition dim)
CH = BC // SUB            # 16 chunks per core
NSUB = 4                  # subtiles per block
BLK = SUB * NSUB          # 512 rows per block
NBLK = BC // BLK          # 4 blocks per core
CH_SPLIT = (12, 4)        # chunks per gather half (uneven: big half first)
BLK_SPLIT = ((0, 1, 2), (3,))
NHALF = 2
NG = 2                    # select feature-groups per half
FG = F // NG              # features per group = 13
W17 = D + 1               # emb row: 16 emb + 1 emb_first
J18 = 18                  # select copies 18 els (18th is table pad 0)
XW = NCONT + F * W17      # 455 = X' row width
NSLOT = V // 4            # 256B-slots per feature (4 rows of 64B each)
CH0 = (0, CH_SPLIT[0])    # chunk offset of each half
IDXW = tuple(c * SUB // 16 for c in CH_SPLIT)   # idx words per (f, half)
IDX_COLS = F * sum(IDXW)


def _chunks(total, step=128):
    return [(s, min(step, total - s)) for s in range(0, total, step)]


def build_kernel():
    KCH = _chunks(XW)          # X' K-chunks: 128,128,128,71
    MCH1 = _chunks(H1)         # L1 M-tiles == L2 K-chunks
    MCH2 = _chunks(H2)         # L2 M-tiles == out-layer K-chunks
    n_wo_ch = len(MCH2)

    nc = bacc.Bacc("TRN2", target_bir_lowering=False, debug=False,
                   num_swdge_queues=4)

    t_tab = nc.dram_tensor("tab", [F * NSLOT, 128], BF16, kind="ExternalInput")
    t_idx = nc.dram_tensor("idx", [128, IDX_COLS], I16, kind="ExternalInput")
    t_msk = nc.dram_tensor("msk", [128, F * CH * 3], U8, kind="ExternalInput")
    t_cont = nc.dram_tensor("cont", [128, CH * NCONT], BF16, kind="ExternalInput")
    t_w1 = nc.dram_tensor("w1p", [128, len(KCH) * H1], BF16, kind="ExternalInput")
    t_w2 = nc.dram_tensor("w2", [128, len(MCH1) * H2], BF16, kind="ExternalInput")
    t_b1 = nc.dram_tensor("b1", [128, len(MCH1)], F32, kind="ExternalInput")
    t_b2 = nc.dram_tensor("b2", [128, len(MCH2)], F32, kind="ExternalInput")
    t_wo = nc.dram_tensor("wo", [128, n_wo_ch], BF16, kind="ExternalInput")
    t_wc = nc.dram_tensor("wc", [128, NCONT], BF16, kind="ExternalInput")
    t_fs4 = nc.dram_tensor("fs4", [128, 4], F32, kind="ExternalInput")
    t_ob = nc.dram_tensor("ob", [1, 1], F32, kind="ExternalInput")
    t_y = nc.dram_tensor("y", [NBLK, 1, BLK], F32, kind="ExternalOutput")

    with tile.TileContext(nc) as tc:
        with (
            tc.tile_pool(name="wpool", bufs=1) as wpool,
            tc.tile_pool(name="gpool", bufs=1) as gpool,
            tc.tile_pool(name="cpool", bufs=1) as cpool,
            tc.tile_pool(name="txpool", bufs=2) as txpool,
            tc.tile_pool(name="xpool", bufs=2) as xpool,
            tc.tile_pool(name="hpool", bufs=2) as hpool,
            tc.tile_pool(name="opool", bufs=2) as opool,
            tc.tile_pool(name="pt_ps", bufs=2, space="PSUM") as pt_ps,
            tc.tile_pool(name="mm1_ps", bufs=3, space="PSUM") as mm1_ps,
            tc.tile_pool(name="mm2_ps", bufs=2, space="PSUM") as mm2_ps,
            tc.tile_pool(name="o_ps", bufs=1, space="PSUM") as o_ps,
        ):
            # ---- idx on the Activation HWDGE stream so gathers are not
            # gated by the bulk weight loads on the sync stream ----
            idx_sb = wpool.tile([128, IDX_COLS], I16)
            col = 0
            idx_off = []
            for h in range(NHALF):
                idx_off.append(col)
                w = F * IDXW[h]
                nc.scalar.dma_start(
                    out=idx_sb[:, col : col + w], in_=t_idx[:, col : col + w])
                col += w
            msk_sb = wpool.tile([128, F * CH * 3], U8)
            nc.sync.dma_start(out=msk_sb[:], in_=t_msk[:])

            ident = wpool.tile([128, 128], BF16)
            make_identity(nc, ident)
            identf = wpool.tile([128, 128], F32)
            make_identity(nc, identf)

            w1all = wpool.tile([128, len(KCH) * H1], BF16)
            nc.sync.dma_start(out=w1all[:], in_=t_w1[:])
            w2all = wpool.tile([128, len(MCH1) * H2], BF16)
            nc.sync.dma_start(out=w2all[:], in_=t_w2[:])
            b1t = wpool.tile([128, len(MCH1)], F32)
            nc.sync.dma_start(out=b1t[:], in_=t_b1[:])
            b2t = wpool.tile([128, len(MCH2)], F32)
            nc.sync.dma_start(out=b2t[:], in_=t_b2[:])
            wo_sb = wpool.tile([128, n_wo_ch], BF16)
            nc.sync.dma_start(out=wo_sb[:], in_=t_wo[:])
            wc_sb = wpool.tile([128, NCONT], BF16)
            nc.sync.dma_start(out=wc_sb[:], in_=t_wc[:])
            fs4_sb = wpool.tile([128, 4], F32)
            nc.sync.dma_start(out=fs4_sb[:], in_=t_fs4[:])
            ob_sb = wpool.tile([1, 1], F32)
            nc.sync.dma_start(out=ob_sb[:], in_=t_ob[:])

            # ---- X' tile for the whole core: [p, ch, 455] bf16 ----
            xp = wpool.tile([128, CH * XW], BF16)
            xp_ap = xp[:]
            nc.sync.dma_start(
                out=bass.AP(tensor=xp.tensor, offset=xp_ap.offset,
                            ap=[xp_ap.ap[0], [XW, CH], [1, NCONT]]),
                in_=t_cont[:].rearrange("p (c w) -> p c w", w=NCONT),
            )

            fmv_h = []

            def _emit_block(blk, h):
                # transpose X' -> xT chunks [128, 512] bf16
                xt_sb = []
                for ci, (k0, ks) in enumerate(KCH):
                    pt = pt_ps.tile([128, BLK], BF16, tag="pt")
                    for s in range(NSUB):
                        ch = blk * NSUB + s
                        nc.tensor.transpose(
                            out=pt[0:ks, s * SUB : (s + 1) * SUB],
                            in_=bass.AP(tensor=xp.tensor,
                                        offset=xp_ap.offset + ch * XW + k0,
                                        ap=[xp_ap.ap[0], [1, ks]]),
                            identity=ident[:],
                        )
                    xt = xpool.tile([128, BLK], BF16, tag=f"xt{ci}")
                    nc.scalar.copy(out=xt[0:ks, :], in_=pt[0:ks, :])
                    xt_sb.append(xt)

                # L1: h1^T = relu(W1'^T X'^T + b1)
                h1_sb = []
                for mi, (m0, ms) in enumerate(MCH1):
                    ps1 = mm1_ps.tile([128, BLK], F32, tag="ps1")
                    for ci, (k0, ks) in enumerate(KCH):
                        nc.tensor.matmul(
                            out=ps1[0:ms, :],
                            lhsT=w1all[0:ks, ci * H1 + m0 : ci * H1 + m0 + ms],
                            rhs=xt_sb[ci][0:ks, :],
                            start=(ci == 0), stop=(ci == len(KCH) - 1),
                        )
                    h1m = hpool.tile([128, BLK], BF16, tag=f"h1m{mi}")
                    nc.scalar.activation(
                        out=h1m[0:ms, :], in_=ps1[0:ms, :], func=AF.Relu,
                        bias=b1t[0:ms, mi : mi + 1],
                    )
                    h1_sb.append(h1m)

                # L2: h2^T = relu(W2^T h1^T + b2)
                h2_sb = []
                for mi, (m0, ms) in enumerate(MCH2):
                    ps2 = mm2_ps.tile([128, BLK], F32, tag="ps2")
                    for ci, (k0, ks) in enumerate(MCH1):
                        nc.tensor.matmul(
                            out=ps2[0:ms, :],
                            lhsT=w2all[0:ks, ci * H2 + m0 : ci * H2 + m0 + ms],
                            rhs=h1_sb[ci][0:ks, :],
                            start=(ci == 0), stop=(ci == len(MCH1) - 1),
                        )
                    h2m = hpool.tile([128, BLK], BF16, tag=f"h2m{mi}")
                    nc.scalar.activation(
                        out=h2m[0:ms, :], in_=ps2[0:ms, :], func=AF.Relu,
                        bias=b2t[0:ms, mi : mi + 1],
                    )
                    h2_sb.append(h2m)

                # out: y = W_out[1:]^T h2^T + fm + b; fm accumulated into the
                # same PSUM via identity-matmul of fmv columns
                pso = o_ps.tile([1, BLK], F32, tag="pso")
                for ci, (k0, ks) in enumerate(MCH2):
                    nc.tensor.matmul(
                        out=pso[0:1, :],
                        lhsT=wo_sb[0:ks, ci : ci + 1],
                        rhs=h2_sb[ci][0:ks, :],
                        start=(ci == 0), stop=False,
                    )
                fmv = fmv_h[h]
                for s in range(NSUB):
                    col = blk * NSUB + s - CH0[h]
                    nc.tensor.matmul(
                        out=pso[0:1, s * SUB : (s + 1) * SUB],
                        lhsT=fmv[:, col : col + 1],
                        rhs=identf[:],
                        start=False, stop=True,
                    )
                orow = opool.tile([1, BLK], F32, tag="orow")
                nc.scalar.activation(
                    out=orow[:], in_=pso[0:1, :], func=AF.Identity,
                    bias=ob_sb[0:1, :],
                )
                nc.sync.dma_start(out=t_y[blk], in_=orow[:])

            for h in range(NHALF):
                CHH = CH_SPLIT[h]
                NIDX = CHH * SUB
                KG = FG * CHH
                # ---- gather: dma_gather per feature (sub-launches of <=1024
                # idxs: the ucode misbehaves beyond 1024), 4 queues ----
                g = gpool.tile([128, F * NIDX], BF16, tag=f"G{h}")
                qn = 0
                for f in range(F):
                    off = 0
                    while off < NIDX:
                        n = min(1024, NIDX - off)
                        nc.gpsimd.dma_gather(
                            out_ap=g[
                                :, f * NIDX + off : f * NIDX + off + n
                            ].rearrange("p (c u) -> p c u", u=128),
                            in_ap=t_tab[f * NSLOT : (f + 1) * NSLOT, :],
                            idxs_ap=idx_sb[
                                :, idx_off[h] + f * IDXW[h] + off // 16
                                   : idx_off[h] + f * IDXW[h] + (off + n) // 16
                            ],
                            num_idxs=n,
                            num_idxs_reg=n,
                            elem_size=128,
                            queue_num=qn % 4,
                        )
                        qn += 1
                        off += n

                g_ap = g[:]
                m_base = msk_sb[:].offset + CH0[h] * F * 3
                tx_grp = []
                for grp in range(NG):
                    # class select for features [grp*FG, (grp+1)*FG):
                    # tx[p, kg, 0:18] = g[p, grp*KG*128 + 128*kg + 32c + j]
                    def g_slice(c):
                        return bass.AP(
                            tensor=g.tensor,
                            offset=g_ap.offset + grp * KG * 128 + 32 * c,
                            ap=[g_ap.ap[0], [128, KG], [1, J18]])

                    m_off = m_base + grp * KG * 3

                    def m_slice(ci):
                        return bass.AP(
                            tensor=msk_sb.tensor, offset=m_off + ci,
                            ap=[msk_sb[:].ap[0], [3, KG], [0, J18]])

                    tx = txpool.tile([128, KG * J18], BF16, tag=f"ctx{h}")
                    tx3 = tx[:].rearrange("p (k j) -> p k j", j=J18)
                    nc.vector.tensor_copy(tx3, g_slice(0))
                    for c in (1, 2, 3):
                        nc.vector.copy_predicated(
                            out=tx3, mask=m_slice(c - 1), data=g_slice(c))
                    tx_grp.append(tx)

                    # reorder (f-major in group) -> X'[p, ch, 13+17f+j]
                    tx_ap = tx[:]
                    src = bass.AP(tensor=tx.tensor, offset=tx_ap.offset,
                                  ap=[tx_ap.ap[0], [J18, CHH],
                                      [CHH * J18, FG], [1, W17]])
                    dst = bass.AP(
                        tensor=xp.tensor,
                        offset=(xp_ap.offset + CH0[h] * XW + NCONT
                                + grp * FG * W17),
                        ap=[xp_ap.ap[0], [XW, CHH], [W17, FG], [1, W17]])
                    nc.scalar.copy(out=dst, in_=src)

                # ---- FM terms from the feature-major tx tiles ----
                seg, r2g, rfg = [], [], []
                for grp in range(NG):
                    tx_ap = tx_grp[grp][:]
                    txt = tx_grp[grp].tensor
                    se_p = cpool.tile([128, CHH * D], F32, name=f"se{grp}h{h}")
                    nc.vector.tensor_reduce(
                        out=se_p[:].rearrange("p (c d) -> p c d", d=D),
                        in_=bass.AP(tensor=txt, offset=tx_ap.offset,
                                    ap=[tx_ap.ap[0], [J18, CHH], [1, D],
                                        [CHH * J18, FG]]),
                        axis=mybir.AxisListType.X, op=ALU.add,
                    )
                    seg.append(se_p)
                    sq = cpool.tile([128, KG * J18], BF16, name=f"sq{grp}h{h}")
                    nc.vector.tensor_mul(out=sq[:], in0=tx_ap, in1=tx_ap)
                    r2_p = cpool.tile([128, CHH], F32, name=f"r2{grp}h{h}")
                    nc.vector.tensor_reduce(
                        out=r2_p[:],
                        in_=bass.AP(tensor=sq.tensor, offset=sq[:].offset,
                                    ap=[sq[:].ap[0], [J18, CHH],
                                        [CHH * J18, FG], [1, D]]),
                        axis=mybir.AxisListType.XY, op=ALU.add,
                    )
                    r2g.append(r2_p)
                    rf_p = cpool.tile([128, CHH], F32, name=f"rf{grp}h{h}")
                    nc.vector.tensor_reduce(
                        out=rf_p[:],
                        in_=bass.AP(tensor=txt, offset=tx_ap.offset + D,
                                    ap=[tx_ap.ap[0], [J18, CHH],
                                        [CHH * J18, FG]]),
                        axis=mybir.AxisListType.X, op=ALU.add,
                    )
                    rfg.append(rf_p)

                se = cpool.tile([128, CHH * D], F32, name=f"seh{h}")
                nc.vector.tensor_add(out=se[:], in0=seg[0][:], in1=seg[1][:])
                se2 = cpool.tile([128, CHH * D], F32, name=f"se2h{h}")
                nc.vector.tensor_mul(out=se2[:], in0=se[:], in1=se[:])
                rr = cpool.tile([128, CHH * 4], F32, name=f"rrh{h}")
                rr_ap = rr[:]

                def rr_slice(idx):
                    return bass.AP(tensor=rr.tensor, offset=rr_ap.offset + idx,
                                   ap=[rr_ap.ap[0], [4, CHH]])

                nc.vector.tensor_reduce(
                    out=rr_slice(0),
                    in_=se2[:].rearrange("p (c d) -> p c d", d=D),
                    axis=mybir.AxisListType.X, op=ALU.add,
                )
                nc.vector.tensor_add(out=rr_slice(1), in0=r2g[0][:],
                                     in1=r2g[1][:])
                nc.vector.tensor_add(out=rr_slice(2), in0=rfg[0][:],
                                     in1=rfg[1][:])
                # r3 = cont . W_cont
                cw = cpool.tile([128, CHH * NCONT], F32, name=f"cwh{h}")
                nc.vector.tensor_mul(
                    out=cw[:].rearrange("p (c w) -> p c w", w=NCONT),
                    in0=bass.AP(tensor=xp.tensor,
                                offset=xp_ap.offset + CH0[h] * XW,
                                ap=[xp_ap.ap[0], [XW, CHH], [1, NCONT]]),
                    in1=bass.AP(tensor=wc_sb.tensor, offset=wc_sb[:].offset,
                                ap=[wc_sb[:].ap[0], [0, CHH], [1, NCONT]]),
                )
                nc.vector.tensor_reduce(
                    out=rr_slice(3),
                    in_=cw[:].rearrange("p (c w) -> p c w", w=NCONT),
                    axis=mybir.AxisListType.X, op=ALU.add,
                )
                # fmv = w_fm * (0.5 r1 - 0.5 r2 + rf + r3)
                ft = cpool.tile([128, CHH * 4], F32, name=f"fth{h}")
                nc.vector.tensor_mul(
                    out=ft[:].rearrange("p (c k) -> p c k", k=4),
                    in0=rr[:].rearrange("p (c k) -> p c k", k=4),
                    in1=bass.AP(tensor=fs4_sb.tensor, offset=fs4_sb[:].offset,
                                ap=[fs4_sb[:].ap[0], [0, CHH], [1, 4]]),
                )
                fmv = cpool.tile([128, CHH], F32, name=f"fmvh{h}")
                nc.vector.tensor_reduce(
                    out=fmv[:],
                    in_=ft[:].rearrange("p (c k) -> p c k", k=4),
                    axis=mybir.AxisListType.X, op=ALU.add,
                )
                fmv_h.append(fmv)

                # ---- MLP for this half's blocks ----
                for blk in BLK_SPLIT[h]:
                    _emit_block(blk, h)

    nc.compile()
    return nc


def prep_inputs(continuous, cat_idx, W_cont, b_cont, emb_first, emb, W1, b1,
                W2, b2, W_out, b_out):
    """Host-side: padded bf16 table, wrapped int16 slot indices, class masks,
    bf16 weights, per-core shards."""
    KCH = _chunks(XW)
    MCH1 = _chunks(H1)
    MCH2 = _chunks(H2)

    # padded table: 4 rows of 64B per 256B slot; row = 16 emb + first + pad
    emb = np.asarray(emb, np.float32)                     # [F, V, D]
    emb_first = np.asarray(emb_first, np.float32)         # [F, V]
    tab = np.zeros((F, NSLOT, 4, 32), BF)
    tab[..., :D] = emb.reshape(F, NSLOT, 4, D).astype(BF)
    tab[..., D] = emb_first.reshape(F, NSLOT, 4).astype(BF)
    tab = np.ascontiguousarray(tab.reshape(F * NSLOT, 128))

    r_all = np.asarray(cat_idx).astype(np.int64)          # [B, F]

    W1 = np.asarray(W1, np.float32)
    w1p = np.zeros((XW, H1), np.float32)
    w1p[0:NCONT] = W1[0:NCONT]
    for f in range(F):
        w1p[NCONT + W17 * f : NCONT + W17 * f + D] = (
            W1[NCONT + D * f : NCONT + D * f + D])
    w1pk = np.zeros((128, len(KCH) * H1), BF)
    for ci, (k0, ks) in enumerate(KCH):
        w1pk[0:ks, ci * H1 : (ci + 1) * H1] = w1p[k0 : k0 + ks].astype(BF)

    W2 = np.asarray(W2, np.float32)
    w2k = np.zeros((128, len(MCH1) * H2), BF)
    for ci, (k0, ks) in enumerate(MCH1):
        w2k[0:ks, ci * H2 : (ci + 1) * H2] = W2[k0 : k0 + ks].astype(BF)

    b1 = np.asarray(b1, np.float32)
    b1t = np.zeros((128, len(MCH1)), np.float32)
    for mi, (m0, ms) in enumerate(MCH1):
        b1t[0:ms, mi] = b1[m0 : m0 + ms]
    b2 = np.asarray(b2, np.float32)
    b2t = np.zeros((128, len(MCH2)), np.float32)
    for mi, (m0, ms) in enumerate(MCH2):
        b2t[0:ms, mi] = b2[m0 : m0 + ms]

    W_out = np.asarray(W_out, np.float32)
    n_wo_ch = (H2 + 127) // 128
    wo_t = np.zeros((n_wo_ch, 128), np.float32)
    wo_t.reshape(-1)[:H2] = W_out[1:, 0]
    wo = np.ascontiguousarray(wo_t.T).astype(BF)

    w_fm = np.float32(W_out[0, 0])
    ob = np.float32(b_out[0] + w_fm * b_cont[0])
    fs4 = np.tile(
        np.array([0.5 * w_fm, -0.5 * w_fm, w_fm, w_fm], np.float32), (128, 1))

    common = {
        "tab": tab,
        "w1p": w1pk,
        "w2": w2k,
        "b1": b1t,
        "b2": b2t,
        "wo": wo,
        "wc": np.tile(np.asarray(W_cont, np.float32).reshape(1, NCONT),
                      (128, 1)).astype(BF),
        "fs4": fs4,
        "ob": np.array([[ob]], np.float32),
    }

    continuous = np.asarray(continuous, np.float32)
    in_maps = []
    for c in range(NCORES):
        rows = slice(c * BC, (c + 1) * BC)
        r = r_all[rows]                                   # [2048, F]
        q = (r >> 2).astype(np.int16)
        cls = (r & 3).astype(np.int64)

        # idx: per (half, f): flat gather i -> sample CH0[h]*128 + i,
        # wrapped i -> [i%16, i//16], replicated to 128 partitions
        idx = np.zeros((16, IDX_COLS), np.int16)
        col = 0
        for h in range(NHALF):
            n = CH_SPLIT[h] * SUB
            qs = q[CH0[h] * SUB : CH0[h] * SUB + n]       # [n, F]
            w = qs.reshape(n // 16, 16, F).transpose(1, 2, 0).reshape(
                16, F * (n // 16))
            idx[:, col : col + w.shape[1]] = w
            col += w.shape[1]
        idx = np.tile(idx, (8, 1))

        # class masks for copy_predicated, laid out per half:
        # col = CH0[h]*F*3 + (f*CHH + ch_h)*3 + (c-1)
        msk = np.zeros((SUB, F * CH * 3), np.uint8)
        for h in range(NHALF):
            chh = CH_SPLIT[h]
            cls_h = cls.reshape(CH, SUB, F)[CH0[h] : CH0[h] + chh]
            onehot = (cls_h[..., None] == np.arange(1, 4)).astype(np.uint8)
            # [ch_h, p, f, c] -> [p, f, ch_h, c]
            msk[:, CH0[h] * F * 3 : (CH0[h] + chh) * F * 3] = (
                onehot.transpose(1, 2, 0, 3).reshape(SUB, chh * F * 3))

        cont = np.ascontiguousarray(
            continuous[rows].reshape(CH, SUB, NCONT)
            .transpose(1, 0, 2).reshape(SUB, CH * NCONT)).astype(BF)

        in_maps.append({**common, "idx": idx, "msk": msk, "cont": cont})
    return in_maps


_NC_CACHE = {}


def kernel(**inputs) -> np.ndarray:
    if "nc" not in _NC_CACHE:
        _NC_CACHE["nc"] = build_kernel()
    nc = _NC_CACHE["nc"]
    in_maps = prep_inputs(**inputs)
    res = run_bass_kernel_spmd(nc, in_maps, core_ids=list(range(NCORES)))
    out = np.concatenate(
        [r["y"].reshape(BC, 1) for r in res.results], axis=0)
    return out.astype(np.float32)


# revision 38
# speedup vs baseline: 1.1606x; 1.1606x over previous
"""DeepFM (embedding_lookup) Trainium2 Bass kernel.

Sharding: data-parallel on batch across 8 NeuronCores; the embedding
table is replicated per core in bf16, padded to 64B rows packed 4-per-
256B slot so the SWDGE dma_gather ucode (int16 slot indices, 256B-
multiple stride) can fetch each sample-feature row as one descriptor.
Gathers are spread over 4 SWDGE queues (4 Q7 cpu pairs generate
descriptors concurrently). A predicated-copy tree on the vector engine
selects the right 64B row out of each 256B slot, split by feature
group so selection overlaps the remaining gathers. FM statistics are
computed from the pre-reorder (feature-major) layout so only the
transposes wait on the X' reorder. The batch is gathered in two uneven
halves (12 chunks + 4 chunks) so most of the MLP overlaps the second
gather wave and only one block remains after the last gather. MLP runs
in bf16 with fp32 PSUM; the FM row is accumulated straight into the
output-layer PSUM via identity matmuls.

Self-contained: hardcodes all shapes from the problem spec.
"""

import numpy as np
import ml_dtypes

import concourse.bass as bass
import concourse.bacc as bacc
import concourse.mybir as mybir
import concourse.tile as tile
from concourse.bass_utils import run_bass_kernel_spmd
from concourse.masks import make_identity

F32 = mybir.dt.float32
BF16 = mybir.dt.bfloat16
I16 = mybir.dt.int16
U8 = mybir.dt.uint8
AF = mybir.ActivationFunctionType
ALU = mybir.AluOpType
BF = ml_dtypes.bfloat16

# Problem dims
B, NCONT, F, V, D = 16384, 13, 26, 100000, 16
H1, H2 = 400, 400
NCORES = 8
BC = B // NCORES          # 2048 rows per core
SUB = 128                 # batch subtile (partition dim)
CH = BC // SUB            # 16 chunks per core
NSUB = 4                  # subtiles per block
BLK = SUB * NSUB          # 512 rows per block
NBLK = BC // BLK          # 4 blocks per core
CH_SPLIT = (8, 8)         # chunks per gather half
BLK_SPLIT = ((0, 1), (2, 3))
NHALF = 2
NG = 2                    # select feature-groups per half
FG = F // NG              # features per group = 13
W17 = D + 1               # emb row: 16 emb + 1 emb_first
J18 = 18                  # select copies 18 els (18th is table pad 0)
XW = NCONT + F * W17      # 455 = X' row width
NSLOT = V // 4            # 256B-slots per feature (4 rows of 64B each)
CH0 = (0, CH_SPLIT[0])    # chunk offset of each half
IDXW = tuple(c * SUB // 16 for c in CH_SPLIT)   # idx words per (f, half)
IDX_COLS = F * sum(IDXW)


def _chunks(total, step=128):
    return [(s, min(step, total - s)) for s in range(0, total, step)]


def build_kernel():
    KCH = _chunks(XW)          # X' K-chunks: 128,128,128,71
    MCH1 = _chunks(H1)         # L1 M-tiles == L2 K-chunks
    MCH2 = _chunks(H2)         # L2 M-tiles == out-layer K-chunks
    n_wo_ch = len(MCH2)

    nc = bacc.Bacc("TRN2", target_bir_lowering=False, debug=False,
                   num_swdge_queues=4)

    t_tab = nc.dram_tensor("tab", [F * NSLOT, 128], BF16, kind="ExternalInput")
    t_idx = nc.dram_tensor("idx", [128, IDX_COLS], I16, kind="ExternalInput")
    t_msk = nc.dram_tensor("msk", [128, F * CH * 3], U8, kind="ExternalInput")
    t_cont = nc.dram_tensor("cont", [128, CH * NCONT], BF16, kind="ExternalInput")
    t_w1 = nc.dram_tensor("w1p", [128, len(KCH) * H1], BF16, kind="ExternalInput")
    t_w2 = nc.dram_tensor("w2", [128, len(MCH1) * H2], BF16, kind="ExternalInput")
    t_b1 = nc.dram_tensor("b1", [128, len(MCH1)], F32, kind="ExternalInput")
    t_b2 = nc.dram_tensor("b2", [128, len(MCH2)], F32, kind="ExternalInput")
    t_wo = nc.dram_tensor("wo", [128, n_wo_ch], BF16, kind="ExternalInput")
    t_wc = nc.dram_tensor("wc", [128, NCONT], BF16, kind="ExternalInput")
    t_fs4 = nc.dram_tensor("fs4", [128, 4], F32, kind="ExternalInput")
    t_ob = nc.dram_tensor("ob", [1, 1], F32, kind="ExternalInput")
    t_y = nc.dram_tensor("y", [NBLK, 1, BLK], F32, kind="ExternalOutput")

    with tile.TileContext(nc) as tc:
        with (
            tc.tile_pool(name="wpool", bufs=1) as wpool,
            tc.tile_pool(name="gpool", bufs=1) as gpool,
            tc.tile_pool(name="cpool", bufs=1) as cpool,
            tc.tile_pool(name="txpool", bufs=2) as txpool,
            tc.tile_pool(name="xpool", bufs=2) as xpool,
            tc.tile_pool(name="hpool", bufs=2) as hpool,
            tc.tile_pool(name="opool", bufs=2) as opool,
            tc.tile_pool(name="pt_ps", bufs=2, space="PSUM") as pt_ps,
            tc.tile_pool(name="mm1_ps", bufs=3, space="PSUM") as mm1_ps,
            tc.tile_pool(name="mm2_ps", bufs=2, space="PSUM") as mm2_ps,
            tc.tile_pool(name="o_ps", bufs=1, space="PSUM") as o_ps,
        ):
            # ---- idx on the Activation HWDGE stream so gathers are not
            # gated by the bulk weight loads on the sync stream ----
            idx_sb = wpool.tile([128, IDX_COLS], I16)
            col = 0
            idx_off = []
            for h in range(NHALF):
                idx_off.append(col)
                w = F * IDXW[h]
                nc.scalar.dma_start(
                    out=idx_sb[:, col : col + w], in_=t_idx[:, col : col + w])
                col += w
            msk_sb = wpool.tile([128, F * CH * 3], U8)
            nc.sync.dma_start(out=msk_sb[:], in_=t_msk[:])

            ident = wpool.tile([128, 128], BF16)
            make_identity(nc, ident)
            identf = wpool.tile([128, 128], F32)
            make_identity(nc, identf)

            w1all = wpool.tile([128, len(KCH) * H1], BF16)
            nc.sync.dma_start(out=w1all[:], in_=t_w1[:])
            w2all = wpool.tile([128, len(MCH1) * H2], BF16)
            nc.sync.dma_start(out=w2all[:], in_=t_w2[:])
            b1t = wpool.tile([128, len(MCH1)], F32)
            nc.sync.dma_start(out=b1t[:], in_=t_b1[:])
            b2t = wpool.tile([128, len(MCH2)], F32)
            nc.sync.dma_start(out=b2t[:], in_=t_b2[:])
            wo_sb = wpool.tile([128, n_wo_ch], BF16)
            nc.sync.dma_start(out=wo_sb[:], in_=t_wo[:])
            wc_sb = wpool.tile([128, NCONT], BF16)
            nc.sync.dma_start(out=wc_sb[:], in_=t_wc[:])
            fs4_sb = wpool.tile([128, 4], F32)
            nc.sync.dma_start(out=fs4_sb[:], in_=t_fs4[:])
            ob_sb = wpool.tile([1, 1], F32)
            nc.sync.dma_start(out=ob_sb[:], in_=t_ob[:])

            # ---- X' tile for the whole core: [p, ch, 455] bf16 ----
            xp = wpool.tile([128, CH * XW], BF16)
            xp_ap = xp[:]
            nc.sync.dma_start(
                out=bass.AP(tensor=xp.tensor, offset=xp_ap.offset,
                            ap=[xp_ap.ap[0], [XW, CH], [1, NCONT]]),
                in_=t_cont[:].rearrange("p (c w) -> p c w", w=NCONT),
            )

            fmv_h = []

            def _emit_block(blk, h):
                # transpose X' -> xT chunks [128, 512] bf16
                xt_sb = []
                for ci, (k0, ks) in enumerate(KCH):
                    pt = pt_ps.tile([128, BLK], BF16, tag="pt")
                    for s in range(NSUB):
                        ch = blk * NSUB + s
                        nc.tensor.transpose(
                            out=pt[0:ks, s * SUB : (s + 1) * SUB],
                            in_=bass.AP(tensor=xp.tensor,
                                        offset=xp_ap.offset + ch * XW + k0,
                                        ap=[xp_ap.ap[0], [1, ks]]),
                            identity=ident[:],
                        )
                    xt = xpool.tile([128, BLK], BF16, tag=f"xt{ci}")
                    nc.scalar.copy(out=xt[0:ks, :], in_=pt[0:ks, :])
                    xt_sb.append(xt)

                # L1: h1^T = relu(W1'^T X'^T + b1)
                h1_sb = []
                for mi, (m0, ms) in enumerate(MCH1):
                    ps1 = mm1_ps.tile([128, BLK], F32, tag="ps1")
                    for ci, (k0, ks) in enumerate(KCH):
                        nc.tensor.matmul(
                            out=ps1[0:ms, :],
                            lhsT=w1all[0:ks, ci * H1 + m0 : ci * H1 + m0 + ms],
                            rhs=xt_sb[ci][0:ks, :],
                            start=(ci == 0), stop=(ci == len(KCH) - 1),
                        )
                    h1m = hpool.tile([128, BLK], BF16, tag=f"h1m{mi}")
                    nc.scalar.activation(
                        out=h1m[0:ms, :], in_=ps1[0:ms, :], func=AF.Relu,
                        bias=b1t[0:ms, mi : mi + 1],
                    )
                    h1_sb.append(h1m)

                # L2: h2^T = relu(W2^T h1^T + b2)
                h2_sb = []
                for mi, (m0, ms) in enumerate(MCH2):
                    ps2 = mm2_ps.tile([128, BLK], F32, tag="ps2")
                    for ci, (k0, ks) in enumerate(MCH1):
                        nc.tensor.matmul(
                            out=ps2[0:ms, :],
                            lhsT=w2all[0:ks, ci * H2 + m0 : ci * H2 + m0 + ms],
                            rhs=h1_sb[ci][0:ks, :],
                            start=(ci == 0), stop=(ci == len(MCH1) - 1),
                        )
                    h2m = hpool.tile([128, BLK], BF16, tag=f"h2m{mi}")
                    nc.scalar.activation(
                        out=h2m[0:ms, :], in_=ps2[0:ms, :], func=AF.Relu,
                        bias=b2t[0:ms, mi : mi + 1],
                    )
                    h2_sb.append(h2m)

                # out: y = W_out[1:]^T h2^T + fm + b; fm accumulated into the
                # same PSUM via identity-matmul of fmv columns
                pso = o_ps.tile([1, BLK], F32, tag="pso")
                for ci, (k0, ks) in enumerate(MCH2):
                    nc.tensor.matmul(
                        out=pso[0:1, :],
                        lhsT=wo_sb[0:ks, ci : ci + 1],
                        rhs=h2_sb[ci][0:ks, :],
                        start=(ci == 0), stop=False,
                    )
                fmv = fmv_h[h]
                for s in range(NSUB):
                    col = blk * NSUB + s - CH0[h]
                    nc.tensor.matmul(
                        out=pso[0:1, s * SUB : (s + 1) * SUB],
                        lhsT=fmv[:, col : col + 1],
                        rhs=identf[:],
                        start=False, stop=True,
                    )
                orow = opool.tile([1, BLK], F32, tag="orow")
                nc.scalar.activation(
                    out=orow[:], in_=pso[0:1, :], func=AF.Identity,
                    bias=ob_sb[0:1, :],
                )
                nc.sync.dma_start(out=t_y[blk], in_=orow[:])

            for h in range(NHALF):
                CHH = CH_SPLIT[h]
                NIDX = CHH * SUB
                KG = FG * CHH
                # ---- gather: dma_gather per feature (sub-launches of <=1024
                # idxs: the ucode misbehaves beyond 1024), 4 queues ----
                g = gpool.tile([128, F * NIDX], BF16, tag=f"G{h}")
                qn = 0
                for f in range(F):
                    off = 0
                    while off < NIDX:
                        n = min(1024, NIDX - off)
                        nc.gpsimd.dma_gather(
                            out_ap=g[
                                :, f * NIDX + off : f * NIDX + off + n
                            ].rearrange("p (c u) -> p c u", u=128),
                            in_ap=t_tab[f * NSLOT : (f + 1) * NSLOT, :],
                            idxs_ap=idx_sb[
                                :, idx_off[h] + f * IDXW[h] + off // 16
                                   : idx_off[h] + f * IDXW[h] + (off + n) // 16
                            ],
                            num_idxs=n,
                            num_idxs_reg=n,
                            elem_size=128,
                            queue_num=qn % 4,
                        )
                        qn += 1
                        off += n

                g_ap = g[:]
                m_base = msk_sb[:].offset + CH0[h] * F * 3
                tx_grp = []
                for grp in range(NG):
                    # class select for features [grp*FG, (grp+1)*FG):
                    # tx[p, kg, 0:18] = g[p, grp*KG*128 + 128*kg + 32c + j]
                    def g_slice(c):
                        return bass.AP(
                            tensor=g.tensor,
                            offset=g_ap.offset + grp * KG * 128 + 32 * c,
                            ap=[g_ap.ap[0], [128, KG], [1, J18]])

                    m_off = m_base + grp * KG * 3

                    def m_slice(ci):
                        return bass.AP(
                            tensor=msk_sb.tensor, offset=m_off + ci,
                            ap=[msk_sb[:].ap[0], [3, KG], [0, J18]])

                    tx = txpool.tile([128, KG * J18], BF16, tag=f"ctx{h}")
                    tx3 = tx[:].rearrange("p (k j) -> p k j", j=J18)
                    nc.vector.tensor_copy(tx3, g_slice(0))
                    for c in (1, 2, 3):
                        nc.vector.copy_predicated(
                            out=tx3, mask=m_slice(c - 1), data=g_slice(c))
                    tx_grp.append(tx)

                    # reorder (f-major in group) -> X'[p, ch, 13+17f+j]
                    tx_ap = tx[:]
                    src = bass.AP(tensor=tx.tensor, offset=tx_ap.offset,
                                  ap=[tx_ap.ap[0], [J18, CHH],
                                      [CHH * J18, FG], [1, W17]])
                    dst = bass.AP(
                        tensor=xp.tensor,
                        offset=(xp_ap.offset + CH0[h] * XW + NCONT
                                + grp * FG * W17),
                        ap=[xp_ap.ap[0], [XW, CHH], [W17, FG], [1, W17]])
                    nc.scalar.copy(out=dst, in_=src)

                # ---- FM terms from the feature-major tx tiles ----
                seg, r2g, rfg = [], [], []
                for grp in range(NG):
                    tx_ap = tx_grp[grp][:]
                    txt = tx_grp[grp].tensor
                    se_p = cpool.tile([128, CHH * D], F32, name=f"se{grp}h{h}")
                    nc.vector.tensor_reduce(
                        out=se_p[:].rearrange("p (c d) -> p c d", d=D),
                        in_=bass.AP(tensor=txt, offset=tx_ap.offset,
                                    ap=[tx_ap.ap[0], [J18, CHH], [1, D],
                                        [CHH * J18, FG]]),
                        axis=mybir.AxisListType.X, op=ALU.add,
                    )
                    seg.append(se_p)
                    sq = cpool.tile([128, KG * J18], BF16, name=f"sq{grp}h{h}")
                    nc.vector.tensor_mul(out=sq[:], in0=tx_ap, in1=tx_ap)
                    r2_p = cpool.tile([128, CHH], F32, name=f"r2{grp}h{h}")
                    nc.vector.tensor_reduce(
                        out=r2_p[:],
                        in_=bass.AP(tensor=sq.tensor, offset=sq[:].offset,
                                    ap=[sq[:].ap[0], [J18, CHH],
                                        [CHH * J18, FG], [1, D]]),
                        axis=mybir.AxisListType.XY, op=ALU.add,
                    )
                    r2g.append(r2_p)
                    rf_p = cpool.tile([128, CHH], F32, name=f"rf{grp}h{h}")
                    nc.vector.tensor_reduce(
                        out=rf_p[:],
                        in_=bass.AP(tensor=txt, offset=tx_ap.offset + D,
                                    ap=[tx_ap.ap[0], [J18, CHH],
                                        [CHH * J18, FG]]),
                        axis=mybir.AxisListType.X, op=ALU.add,
                    )
                    rfg.append(rf_p)

                se = cpool.tile([128, CHH * D], F32, name=f"seh{h}")
                nc.vector.tensor_add(out=se[:], in0=seg[0][:], in1=seg[1][:])
                se2 = cpool.tile([128, CHH * D], F32, name=f"se2h{h}")
                nc.vector.tensor_mul(out=se2[:], in0=se[:], in1=se[:])
                rr = cpool.tile([128, CHH * 4], F32, name=f"rrh{h}")
                rr_ap = rr[:]

                def rr_slice(idx):
                    return bass.AP(tensor=rr.tensor, offset=rr_ap.offset + idx,
                                   ap=[rr_ap.ap[0], [4, CHH]])

                nc.vector.tensor_reduce(
                    out=rr_slice(0),
                    in_=se2[:].rearrange("p (c d) -> p c d", d=D),
                    axis=mybir.AxisListType.X, op=ALU.add,
                )
                nc.vector.tensor_add(out=rr_slice(1), in0=r2g[0][:],
                                     in1=r2g[1][:])
                nc.vector.tensor_add(out=rr_slice(2), in0=rfg[0][:],
                                     in1=rfg[1][:])
                # r3 = cont . W_cont
                cw = cpool.tile([128, CHH * NCONT], F32, name=f"cwh{h}")
                nc.vector.tensor_mul(
                    out=cw[:].rearrange("p (c w) -> p c w", w=NCONT),
                    in0=bass.AP(tensor=xp.tensor,
                                offset=xp_ap.offset + CH0[h] * XW,
                                ap=[xp_ap.ap[0], [XW, CHH], [1, NCONT]]),
                    in1=bass.AP(tensor=wc_sb.tensor, offset=wc_sb[:].offset,
                                ap=[wc_sb[:].ap[0], [0, CHH], [1, NCONT]]),
                )
                nc.vector.tensor_reduce(
                    out=rr_slice(3),
                    in_=cw[:].rearrange("p (c w) -> p c w", w=NCONT),
                    axis=mybir.AxisListType.X, op=ALU.add,
                )
                # fmv = w_fm * (0.5 r1 - 0.5 r2 + rf + r3)
                ft = cpool.tile([128, CHH * 4], F32, name=f"fth{h}")
                nc.vector.tensor_mul(
                    out=ft[:].rearrange("p (c k) -> p c k", k=4),
                    in0=rr[:].rearrange("p (c k) -> p c k", k=4),
                    in1=bass.AP(tensor=fs4_sb.tensor, offset=fs4_sb[:].offset,
                                ap=[fs4_sb[:].ap[0], [0, CHH], [1, 4]]),
                )
                fmv = cpool.tile([128, CHH], F32, name=f"fmvh{h}")
                nc.vector.tensor_reduce(
                    out=fmv[:],
                    in_=ft[:].rearrange("p (c k) -> p c k", k=4),
                    axis=mybir.AxisListType.X, op=ALU.add,
                )
                fmv_h.append(fmv)

                # ---- MLP for this half's blocks ----
                for blk in BLK_SPLIT[h]:
                    _emit_block(blk, h)

    nc.compile()
    return nc


def prep_inputs(continuous, cat_idx, W_cont, b_cont, emb_first, emb, W1, b1,
                W2, b2, W_out, b_out):
    """Host-side: padded bf16 table, wrapped int16 slot indices, class masks,
    bf16 weights, per-core shards."""
    KCH = _chunks(XW)
    MCH1 = _chunks(H1)
    MCH2 = _chunks(H2)

    # padded table: 4 rows of 64B per 256B slot; row = 16 emb + first + pad
    emb = np.asarray(emb, np.float32)                     # [F, V, D]
    emb_first = np.asarray(emb_first, np.float32)         # [F, V]
    tab = np.zeros((F, NSLOT, 4, 32), BF)
    tab[..., :D] = emb.reshape(F, NSLOT, 4, D).astype(BF)
    tab[..., D] = emb_first.reshape(F, NSLOT, 4).astype(BF)
    tab = np.ascontiguousarray(tab.reshape(F * NSLOT, 128))

    r_all = np.asarray(cat_idx).astype(np.int64)          # [B, F]

    W1 = np.asarray(W1, np.float32)
    w1p = np.zeros((XW, H1), np.float32)
    w1p[0:NCONT] = W1[0:NCONT]
    for f in range(F):
        w1p[NCONT + W17 * f : NCONT + W17 * f + D] = (
            W1[NCONT + D * f : NCONT + D * f + D])
    w1pk = np.zeros((128, len(KCH) * H1), BF)
    for ci, (k0, ks) in enumerate(KCH):
        w1pk[0:ks, ci * H1 : (ci + 1) * H1] = w1p[k0 : k0 + ks].astype(BF)

    W2 = np.asarray(W2, np.float32)
    w2k = np.zeros((128, len(MCH1) * H2), BF)
    for ci, (k0, ks) in enumerate(MCH1):
        w2k[0:ks, ci * H2 : (ci + 1) * H2] = W2[k0 : k0 + ks].astype(BF)

    b1 = np.asarray(b1, np.float32)
    b1t = np.zeros((128, len(MCH1)), np.float32)
    for mi, (m0, ms) in enumerate(MCH1):
        b1t[0:ms, mi] = b1[m0 : m0 + ms]
    b2 = np.asarray(b2, np.float32)
    b2t = np.zeros((128, len(MCH2)), np.float32)
    for mi, (m0, ms) in enumerate(MCH2):
        b2t[0:ms, mi] = b2[m0 : m0 + ms]

    W_out = np.asarray(W_out, np.float32)
    n_wo_ch = (H2 + 127) // 128
    wo_t = np.zeros((n_wo_ch, 128), np.float32)
    wo_t.reshape(-1)[:H2] = W_out[1:, 0]
    wo = np.ascontiguousarray(wo_t.T).astype(BF)

    w_fm = np.float32(W_out[0, 0])
    ob = np.float32(b_out[0] + w_fm * b_cont[0])
    fs4 = np.tile(
        np.array([0.5 * w_fm, -0.5 * w_fm, w_fm, w_fm], np.float32), (128, 1))

    common = {
        "tab": tab,
        "w1p": w1pk,
        "w2": w2k,
        "b1": b1t,
        "b2": b2t,
        "wo": wo,
        "wc": np.tile(np.asarray(W_cont, np.float32).reshape(1, NCONT),
                      (128, 1)).astype(BF),
        "fs4": fs4,
        "ob": np.array([[ob]], np.float32),
    }

    continuous = np.asarray(continuous, np.float32)
    in_maps = []
    for c in range(NCORES):
        rows = slice(c * BC, (c + 1) * BC)
        r = r_all[rows]                                   # [2048, F]
        q = (r >> 2).astype(np.int16)
        cls = (r & 3).astype(np.int64)

        # idx: per (half, f): flat gather i -> sample CH0[h]*128 + i,
        # wrapped i -> [i%16, i//16], replicated to 128 partitions
        idx = np.zeros((16, IDX_COLS), np.int16)
        col = 0
        for h in range(NHALF):
            n = CH_SPLIT[h] * SUB
            qs = q[CH0[h] * SUB : CH0[h] * SUB + n]       # [n, F]
            w = qs.reshape(n // 16, 16, F).transpose(1, 2, 0).reshape(
                16, F * (n // 16))
            idx[:, col : col + w.shape[1]] = w
            col += w.shape[1]
        idx = np.tile(idx, (8, 1))

        # class masks for copy_predicated, laid out per half:
        # col = CH0[h]*F*3 + (f*CHH + ch_h)*3 + (c-1)
        msk = np.zeros((SUB, F * CH * 3), np.uint8)
        for h in range(NHALF):
            chh = CH_SPLIT[h]
            cls_h = cls.reshape(CH, SUB, F)[CH0[h] : CH0[h] + chh]
            onehot = (cls_h[..., None] == np.arange(1, 4)).astype(np.uint8)
            # [ch_h, p, f, c] -> [p, f, ch_h, c]
            msk[:, CH0[h] * F * 3 : (CH0[h] + chh) * F * 3] = (
                onehot.transpose(1, 2, 0, 3).reshape(SUB, chh * F * 3))

        cont = np.ascontiguousarray(
            continuous[rows].reshape(CH, SUB, NCONT)
            .transpose(1, 0, 2).reshape(SUB, CH * NCONT)).astype(BF)

        in_maps.append({**common, "idx": idx, "msk": msk, "cont": cont})
    return in_maps


_NC_CACHE = {}


def kernel(**inputs) -> np.ndarray:
    if "nc" not in _NC_CACHE:
        _NC_CACHE["nc"] = build_kernel()
    nc = _NC_CACHE["nc"]
    in_maps = prep_inputs(**inputs)
    res = run_bass_kernel_spmd(nc, in_maps, core_ids=list(range(NCORES)))
    out = np.concatenate(
        [r["y"].reshape(BC, 1) for r in res.results], axis=0)
    return out.astype(np.float32)


# revision 39
# speedup vs baseline: 1.1622x; 1.0013x over previous
"""DeepFM (embedding_lookup) Trainium2 Bass kernel.

Sharding: data-parallel on batch across 8 NeuronCores; the embedding
table is replicated per core in bf16, padded to 64B rows packed 4-per-
256B slot so the SWDGE dma_gather ucode (int16 slot indices, 256B-
multiple stride) can fetch each sample-feature row as one descriptor.
Gathers are spread over 4 SWDGE queues (4 Q7 cpu pairs generate
descriptors concurrently). A predicated-copy tree on the vector engine
selects the right 64B row out of each 256B slot, split by feature
group so selection overlaps the remaining gathers. FM statistics are
computed from the pre-reorder (feature-major) layout so only the
transposes wait on the X' reorder. The batch is gathered in two uneven
halves (12 chunks + 4 chunks) so most of the MLP overlaps the second
gather wave and only one block remains after the last gather. MLP runs
in bf16 with fp32 PSUM; the FM row is accumulated straight into the
output-layer PSUM via identity matmuls.

Self-contained: hardcodes all shapes from the problem spec.
"""

import numpy as np
import ml_dtypes

import concourse.bass as bass
import concourse.bacc as bacc
import concourse.mybir as mybir
import concourse.tile as tile
from concourse.bass_utils import run_bass_kernel_spmd
from concourse.masks import make_identity

F32 = mybir.dt.float32
BF16 = mybir.dt.bfloat16
I16 = mybir.dt.int16
U8 = mybir.dt.uint8
AF = mybir.ActivationFunctionType
ALU = mybir.AluOpType
BF = ml_dtypes.bfloat16

# Problem dims
B, NCONT, F, V, D = 16384, 13, 26, 100000, 16
H1, H2 = 400, 400
NCORES = 8
BC = B // NCORES          # 2048 rows per core
SUB = 128                 # batch subtile (partition dim)
CH = BC // SUB            # 16 chunks per core
NSUB = 4                  # subtiles per block
BLK = SUB * NSUB          # 512 rows per block
NBLK = BC // BLK          # 4 blocks per core
CH_SPLIT = (8, 8)         # chunks per gather half
BLK_SPLIT = ((0, 1), (2, 3))
NHALF = 2
NG = 2                    # select feature-groups per half
FG = F // NG              # features per group = 13
W17 = D + 1               # emb row: 16 emb + 1 emb_first
J18 = 18                  # select copies 18 els (18th is table pad 0)
XW = NCONT + F * W17      # 455 = X' row width
NSLOT = V // 4            # 256B-slots per feature (4 rows of 64B each)
CH0 = (0, CH_SPLIT[0])    # chunk offset of each half
IDXW = tuple(c * SUB // 16 for c in CH_SPLIT)   # idx words per (f, half)
IDX_COLS = F * sum(IDXW)


def _chunks(total, step=128):
    return [(s, min(step, total - s)) for s in range(0, total, step)]


def build_kernel():
    KCH = _chunks(XW)          # X' K-chunks: 128,128,128,71
    MCH1 = _chunks(H1)         # L1 M-tiles == L2 K-chunks
    MCH2 = _chunks(H2)         # L2 M-tiles == out-layer K-chunks
    n_wo_ch = len(MCH2)

    nc = bacc.Bacc("TRN2", target_bir_lowering=False, debug=False,
                   num_swdge_queues=4)

    t_tab = nc.dram_tensor("tab", [F * NSLOT, 128], BF16, kind="ExternalInput")
    t_idx = nc.dram_tensor("idx", [128, IDX_COLS], I16, kind="ExternalInput")
    t_msk = nc.dram_tensor("msk", [128, F * CH * 3], U8, kind="ExternalInput")
    t_cont = nc.dram_tensor("cont", [128, CH * NCONT], BF16, kind="ExternalInput")
    t_w1 = nc.dram_tensor("w1p", [128, len(KCH) * H1], BF16, kind="ExternalInput")
    t_w2 = nc.dram_tensor("w2", [128, len(MCH1) * H2], BF16, kind="ExternalInput")
    t_b1 = nc.dram_tensor("b1", [128, len(MCH1)], F32, kind="ExternalInput")
    t_b2 = nc.dram_tensor("b2", [128, len(MCH2)], F32, kind="ExternalInput")
    t_wo = nc.dram_tensor("wo", [128, n_wo_ch], BF16, kind="ExternalInput")
    t_wc = nc.dram_tensor("wc", [128, NCONT], BF16, kind="ExternalInput")
    t_fs4 = nc.dram_tensor("fs4", [128, 4], F32, kind="ExternalInput")
    t_ob = nc.dram_tensor("ob", [1, 1], F32, kind="ExternalInput")
    t_y = nc.dram_tensor("y", [NBLK, 1, BLK], F32, kind="ExternalOutput")

    with tile.TileContext(nc) as tc:
        with (
            tc.tile_pool(name="wpool", bufs=1) as wpool,
            tc.tile_pool(name="gpool", bufs=1) as gpool,
            tc.tile_pool(name="cpool", bufs=1) as cpool,
            tc.tile_pool(name="txpool", bufs=2) as txpool,
            tc.tile_pool(name="xpool", bufs=2) as xpool,
            tc.tile_pool(name="hpool", bufs=2) as hpool,
            tc.tile_pool(name="opool", bufs=2) as opool,
            tc.tile_pool(name="pt_ps", bufs=2, space="PSUM") as pt_ps,
            tc.tile_pool(name="mm1_ps", bufs=3, space="PSUM") as mm1_ps,
            tc.tile_pool(name="mm2_ps", bufs=2, space="PSUM") as mm2_ps,
            tc.tile_pool(name="o_ps", bufs=1, space="PSUM") as o_ps,
        ):
            # ---- idx on the Activation HWDGE stream so gathers are not
            # gated by the bulk weight loads on the sync stream ----
            idx_sb = wpool.tile([128, IDX_COLS], I16)
            col = 0
            idx_off = []
            for h in range(NHALF):
                idx_off.append(col)
                w = F * IDXW[h]
                nc.scalar.dma_start(
                    out=idx_sb[:, col : col + w], in_=t_idx[:, col : col + w])
                col += w
            msk_sb = wpool.tile([128, F * CH * 3], U8)
            nc.sync.dma_start(out=msk_sb[:], in_=t_msk[:])

            ident = wpool.tile([128, 128], BF16)
            make_identity(nc, ident)
            identf = wpool.tile([128, 128], F32)
            make_identity(nc, identf)

            w1all = wpool.tile([128, len(KCH) * H1], BF16)
            nc.sync.dma_start(out=w1all[:], in_=t_w1[:])
            w2all = wpool.tile([128, len(MCH1) * H2], BF16)
            nc.sync.dma_start(out=w2all[:], in_=t_w2[:])
            b1t = wpool.tile([128, len(MCH1)], F32)
            nc.sync.dma_start(out=b1t[:], in_=t_b1[:])
            b2t = wpool.tile([128, len(MCH2)], F32)
            nc.sync.dma_start(out=b2t[:], in_=t_b2[:])
            wo_sb = wpool.tile([128, n_wo_ch], BF16)
            nc.sync.dma_start(out=wo_sb[:], in_=t_wo[:])
            wc_sb = wpool.tile([128, NCONT], BF16)
            nc.sync.dma_start(out=wc_sb[:], in_=t_wc[:])
            fs4_sb = wpool.tile([128, 4], F32)
            nc.sync.dma_start(out=fs4_sb[:], in_=t_fs4[:])
            ob_sb = wpool.tile([1, 1], F32)
            nc.sync.dma_start(out=ob_sb[:], in_=t_ob[:])

            # ---- X' tile for the whole core: [p, ch, 455] bf16 ----
            xp = wpool.tile([128, CH * XW], BF16)
            xp_ap = xp[:]
            nc.sync.dma_start(
                out=bass.AP(tensor=xp.tensor, offset=xp_ap.offset,
                            ap=[xp_ap.ap[0], [XW, CH], [1, NCONT]]),
                in_=t_cont[:].rearrange("p (c w) -> p c w", w=NCONT),
            )

            nidx_reg = nc.gpsimd.to_reg(CH_SPLIT[0] * SUB)
            fmv_h = []

            def _emit_block(blk, h):
                # transpose X' -> xT chunks [128, 512] bf16
                xt_sb = []
                for ci, (k0, ks) in enumerate(KCH):
                    pt = pt_ps.tile([128, BLK], BF16, tag="pt")
                    for s in range(NSUB):
                        ch = blk * NSUB + s
                        nc.tensor.transpose(
                            out=pt[0:ks, s * SUB : (s + 1) * SUB],
                            in_=bass.AP(tensor=xp.tensor,
                                        offset=xp_ap.offset + ch * XW + k0,
                                        ap=[xp_ap.ap[0], [1, ks]]),
                            identity=ident[:],
                        )
                    xt = xpool.tile([128, BLK], BF16, tag=f"xt{ci}")
                    nc.scalar.copy(out=xt[0:ks, :], in_=pt[0:ks, :])
                    xt_sb.append(xt)

                # L1: h1^T = relu(W1'^T X'^T + b1)
                h1_sb = []
                for mi, (m0, ms) in enumerate(MCH1):
                    ps1 = mm1_ps.tile([128, BLK], F32, tag="ps1")
                    for ci, (k0, ks) in enumerate(KCH):
                        nc.tensor.matmul(
                            out=ps1[0:ms, :],
                            lhsT=w1all[0:ks, ci * H1 + m0 : ci * H1 + m0 + ms],
                            rhs=xt_sb[ci][0:ks, :],
                            start=(ci == 0), stop=(ci == len(KCH) - 1),
                        )
                    h1m = hpool.tile([128, BLK], BF16, tag=f"h1m{mi}")
                    nc.scalar.activation(
                        out=h1m[0:ms, :], in_=ps1[0:ms, :], func=AF.Relu,
                        bias=b1t[0:ms, mi : mi + 1],
                    )
                    h1_sb.append(h1m)

                # L2: h2^T = relu(W2^T h1^T + b2)
                h2_sb = []
                for mi, (m0, ms) in enumerate(MCH2):
                    ps2 = mm2_ps.tile([128, BLK], F32, tag="ps2")
                    for ci, (k0, ks) in enumerate(MCH1):
                        nc.tensor.matmul(
                            out=ps2[0:ms, :],
                            lhsT=w2all[0:ks, ci * H2 + m0 : ci * H2 + m0 + ms],
                            rhs=h1_sb[ci][0:ks, :],
                            start=(ci == 0), stop=(ci == len(MCH1) - 1),
                        )
                    h2m = hpool.tile([128, BLK], BF16, tag=f"h2m{mi}")
                    nc.scalar.activation(
                        out=h2m[0:ms, :], in_=ps2[0:ms, :], func=AF.Relu,
                        bias=b2t[0:ms, mi : mi + 1],
                    )
                    h2_sb.append(h2m)

                # out: y = W_out[1:]^T h2^T + fm + b; fm accumulated into the
                # same PSUM via identity-matmul of fmv columns
                pso = o_ps.tile([1, BLK], F32, tag="pso")
                for ci, (k0, ks) in enumerate(MCH2):
                    nc.tensor.matmul(
                        out=pso[0:1, :],
                        lhsT=wo_sb[0:ks, ci : ci + 1],
                        rhs=h2_sb[ci][0:ks, :],
                        start=(ci == 0), stop=False,
                    )
                fmv = fmv_h[h]
                for s in range(NSUB):
                    col = blk * NSUB + s - CH0[h]
                    nc.tensor.matmul(
                        out=pso[0:1, s * SUB : (s + 1) * SUB],
                        lhsT=fmv[:, col : col + 1],
                        rhs=identf[:],
                        start=False, stop=True,
                    )
                orow = opool.tile([1, BLK], F32, tag="orow")
                nc.scalar.activation(
                    out=orow[:], in_=pso[0:1, :], func=AF.Identity,
                    bias=ob_sb[0:1, :],
                )
                nc.sync.dma_start(out=t_y[blk], in_=orow[:])

            for h in range(NHALF):
                CHH = CH_SPLIT[h]
                NIDX = CHH * SUB
                KG = FG * CHH
                # ---- gather: dma_gather per feature (sub-launches of <=1024
                # idxs: the ucode misbehaves beyond 1024), 4 queues ----
                g = gpool.tile([128, F * NIDX], BF16, tag=f"G{h}")
                qn = 0
                for f in range(F):
                    off = 0
                    while off < NIDX:
                        n = min(1024, NIDX - off)
                        nc.gpsimd.dma_gather(
                            out_ap=g[
                                :, f * NIDX + off : f * NIDX + off + n
                            ].rearrange("p (c u) -> p c u", u=128),
                            in_ap=t_tab[f * NSLOT : (f + 1) * NSLOT, :],
                            idxs_ap=idx_sb[
                                :, idx_off[h] + f * IDXW[h] + off // 16
                                   : idx_off[h] + f * IDXW[h] + (off + n) // 16
                            ],
                            num_idxs=n,
                            num_idxs_reg=nidx_reg,
                            elem_size=128,
                            queue_num=qn % 4,
                        )
                        qn += 1
                        off += n

                g_ap = g[:]
                m_base = msk_sb[:].offset + CH0[h] * F * 3
                tx_grp = []
                for grp in range(NG):
                    # class select for features [grp*FG, (grp+1)*FG):
                    # tx[p, kg, 0:18] = g[p, grp*KG*128 + 128*kg + 32c + j]
                    def g_slice(c):
                        return bass.AP(
                            tensor=g.tensor,
                            offset=g_ap.offset + grp * KG * 128 + 32 * c,
                            ap=[g_ap.ap[0], [128, KG], [1, J18]])

                    m_off = m_base + grp * KG * 3

                    def m_slice(ci):
                        return bass.AP(
                            tensor=msk_sb.tensor, offset=m_off + ci,
                            ap=[msk_sb[:].ap[0], [3, KG], [0, J18]])

                    tx = txpool.tile([128, KG * J18], BF16, tag=f"ctx{h}")
                    tx3 = tx[:].rearrange("p (k j) -> p k j", j=J18)
                    nc.vector.tensor_copy(tx3, g_slice(0))
                    for c in (1, 2, 3):
                        nc.vector.copy_predicated(
                            out=tx3, mask=m_slice(c - 1), data=g_slice(c))
                    tx_grp.append(tx)

                    # reorder (f-major in group) -> X'[p, ch, 13+17f+j]
                    tx_ap = tx[:]
                    src = bass.AP(tensor=tx.tensor, offset=tx_ap.offset,
                                  ap=[tx_ap.ap[0], [J18, CHH],
                                      [CHH * J18, FG], [1, W17]])
                    dst = bass.AP(
                        tensor=xp.tensor,
                        offset=(xp_ap.offset + CH0[h] * XW + NCONT
                                + grp * FG * W17),
                        ap=[xp_ap.ap[0], [XW, CHH], [W17, FG], [1, W17]])
                    nc.scalar.copy(out=dst, in_=src)

                # ---- FM terms from the feature-major tx tiles ----
                seg, r2g, rfg = [], [], []
                for grp in range(NG):
                    tx_ap = tx_grp[grp][:]
                    txt = tx_grp[grp].tensor
                    se_p = cpool.tile([128, CHH * D], F32, name=f"se{grp}h{h}")
                    nc.vector.tensor_reduce(
                        out=se_p[:].rearrange("p (c d) -> p c d", d=D),
                        in_=bass.AP(tensor=txt, offset=tx_ap.offset,
                                    ap=[tx_ap.ap[0], [J18, CHH], [1, D],
                                        [CHH * J18, FG]]),
                        axis=mybir.AxisListType.X, op=ALU.add,
                    )
                    seg.append(se_p)
                    sq = cpool.tile([128, KG * J18], BF16, name=f"sq{grp}h{h}")
                    nc.vector.tensor_mul(out=sq[:], in0=tx_ap, in1=tx_ap)
                    r2_p = cpool.tile([128, CHH], F32, name=f"r2{grp}h{h}")
                    nc.vector.tensor_reduce(
                        out=r2_p[:],
                        in_=bass.AP(tensor=sq.tensor, offset=sq[:].offset,
                                    ap=[sq[:].ap[0], [J18, CHH],
                                        [CHH * J18, FG], [1, D]]),
                        axis=mybir.AxisListType.XY, op=ALU.add,
                    )
                    r2g.append(r2_p)
                    rf_p = cpool.tile([128, CHH], F32, name=f"rf{grp}h{h}")
                    nc.vector.tensor_reduce(
                        out=rf_p[:],
                        in_=bass.AP(tensor=txt, offset=tx_ap.offset + D,
                                    ap=[tx_ap.ap[0], [J18, CHH],
                                        [CHH * J18, FG]]),
                        axis=mybir.AxisListType.X, op=ALU.add,
                    )
                    rfg.append(rf_p)

                se = cpool.tile([128, CHH * D], F32, name=f"seh{h}")
                nc.vector.tensor_add(out=se[:], in0=seg[0][:], in1=seg[1][:])
                se2 = cpool.tile([128, CHH * D], F32, name=f"se2h{h}")
                nc.vector.tensor_mul(out=se2[:], in0=se[:], in1=se[:])
                rr = cpool.tile([128, CHH * 4], F32, name=f"rrh{h}")
                rr_ap = rr[:]

                def rr_slice(idx):
                    return bass.AP(tensor=rr.tensor, offset=rr_ap.offset + idx,
                                   ap=[rr_ap.ap[0], [4, CHH]])

                nc.vector.tensor_reduce(
                    out=rr_slice(0),
                    in_=se2[:].rearrange("p (c d) -> p c d", d=D),
                    axis=mybir.AxisListType.X, op=ALU.add,
                )
                nc.vector.tensor_add(out=rr_slice(1), in0=r2g[0][:],
                                     in1=r2g[1][:])
                nc.vector.tensor_add(out=rr_slice(2), in0=rfg[0][:],
                                     in1=rfg[1][:])
                # r3 = cont . W_cont
                cw = cpool.tile([128, CHH * NCONT], F32, name=f"cwh{h}")
                nc.vector.tensor_mul(
                    out=cw[:].rearrange("p (c w) -> p c w", w=NCONT),
                    in0=bass.AP(tensor=xp.tensor,
                                offset=xp_ap.offset + CH0[h] * XW,
                                ap=[xp_ap.ap[0], [XW, CHH], [1, NCONT]]),
                    in1=bass.AP(tensor=wc_sb.tensor, offset=wc_sb[:].offset,
                                ap=[wc_sb[:].ap[0], [0, CHH], [1, NCONT]]),
                )
                nc.vector.tensor_reduce(
                    out=rr_slice(3),
                    in_=cw[:].rearrange("p (c w) -> p c w", w=NCONT),
                    axis=mybir.AxisListType.X, op=ALU.add,
                )
                # fmv = w_fm * (0.5 r1 - 0.5 r2 + rf + r3)
                ft = cpool.tile([128, CHH * 4], F32, name=f"fth{h}")
                nc.vector.tensor_mul(
                    out=ft[:].rearrange("p (c k) -> p c k", k=4),
                    in0=rr[:].rearrange("p (c k) -> p c k", k=4),
                    in1=bass.AP(tensor=fs4_sb.tensor, offset=fs4_sb[:].offset,
                                ap=[fs4_sb[:].ap[0], [0, CHH], [1, 4]]),
                )
                fmv = cpool.tile([128, CHH], F32, name=f"fmvh{h}")
                nc.vector.tensor_reduce(
                    out=fmv[:],
                    in_=ft[:].rearrange("p (c k) -> p c k", k=4),
                    axis=mybir.AxisListType.X, op=ALU.add,
                )
                fmv_h.append(fmv)

                # ---- MLP for this half's blocks ----
                for blk in BLK_SPLIT[h]:
                    _emit_block(blk, h)

    nc.compile()
    return nc


def prep_inputs(continuous, cat_idx, W_cont, b_cont, emb_first, emb, W1, b1,
                W2, b2, W_out, b_out):
    """Host-side: padded bf16 table, wrapped int16 slot indices, class masks,
    bf16 weights, per-core shards."""
    KCH = _chunks(XW)
    MCH1 = _chunks(H1)
    MCH2 = _chunks(H2)

    # padded table: 4 rows of 64B per 256B slot; row = 16 emb + first + pad
    emb = np.asarray(emb, np.float32)                     # [F, V, D]
    emb_first = np.asarray(emb_first, np.float32)         # [F, V]
    tab = np.zeros((F, NSLOT, 4, 32), BF)
    tab[..., :D] = emb.reshape(F, NSLOT, 4, D).astype(BF)
    tab[..., D] = emb_first.reshape(F, NSLOT, 4).astype(BF)
    tab = np.ascontiguousarray(tab.reshape(F * NSLOT, 128))

    r_all = np.asarray(cat_idx).astype(np.int64)          # [B, F]

    W1 = np.asarray(W1, np.float32)
    w1p = np.zeros((XW, H1), np.float32)
    w1p[0:NCONT] = W1[0:NCONT]
    for f in range(F):
        w1p[NCONT + W17 * f : NCONT + W17 * f + D] = (
            W1[NCONT + D * f : NCONT + D * f + D])
    w1pk = np.zeros((128, len(KCH) * H1), BF)
    for ci, (k0, ks) in enumerate(KCH):
        w1pk[0:ks, ci * H1 : (ci + 1) * H1] = w1p[k0 : k0 + ks].astype(BF)

    W2 = np.asarray(W2, np.float32)
    w2k = np.zeros((128, len(MCH1) * H2), BF)
    for ci, (k0, ks) in enumerate(MCH1):
        w2k[0:ks, ci * H2 : (ci + 1) * H2] = W2[k0 : k0 + ks].astype(BF)

    b1 = np.asarray(b1, np.float32)
    b1t = np.zeros((128, len(MCH1)), np.float32)
    for mi, (m0, ms) in enumerate(MCH1):
        b1t[0:ms, mi] = b1[m0 : m0 + ms]
    b2 = np.asarray(b2, np.float32)
    b2t = np.zeros((128, len(MCH2)), np.float32)
    for mi, (m0, ms) in enumerate(MCH2):
        b2t[0:ms, mi] = b2[m0 : m0 + ms]

    W_out = np.asarray(W_out, np.float32)
    n_wo_ch = (H2 + 127) // 128
    wo_t = np.zeros((n_wo_ch, 128), np.float32)
    wo_t.reshape(-1)[:H2] = W_out[1:, 0]
    wo = np.ascontiguousarray(wo_t.T).astype(BF)

    w_fm = np.float32(W_out[0, 0])
    ob = np.float32(b_out[0] + w_fm * b_cont[0])
    fs4 = np.tile(
        np.array([0.5 * w_fm, -0.5 * w_fm, w_fm, w_fm], np.float32), (128, 1))

    common = {
        "tab": tab,
        "w1p": w1pk,
        "w2": w2k,
        "b1": b1t,
        "b2": b2t,
        "wo": wo,
        "wc": np.tile(np.asarray(W_cont, np.float32).reshape(1, NCONT),
                      (128, 1)).astype(BF),
        "fs4": fs4,
        "ob": np.array([[ob]], np.float32),
    }

    continuous = np.asarray(continuous, np.float32)
    in_maps = []
    for c in range(NCORES):
        rows = slice(c * BC, (c + 1) * BC)
        r = r_all[rows]                                   # [2048, F]
        q = (r >> 2).astype(np.int16)
        cls = (r & 3).astype(np.int64)

        # idx: per (half, f): flat gather i -> sample CH0[h]*128 + i,
        # wrapped i -> [i%16, i//16], replicated to 128 partitions
        idx = np.zeros((16, IDX_COLS), np.int16)
        col = 0
        for h in range(NHALF):
            n = CH_SPLIT[h] * SUB
            qs = q[CH0[h] * SUB : CH0[h] * SUB + n]       # [n, F]
            w = qs.reshape(n // 16, 16, F).transpose(1, 2, 0).reshape(
                16, F * (n // 16))
            idx[:, col : col + w.shape[1]] = w
            col += w.shape[1]
        idx = np.tile(idx, (8, 1))

        # class masks for copy_predicated, laid out per half:
        # col = CH0[h]*F*3 + (f*CHH + ch_h)*3 + (c-1)
        msk = np.zeros((SUB, F * CH * 3), np.uint8)
        for h in range(NHALF):
            chh = CH_SPLIT[h]
            cls_h = cls.reshape(CH, SUB, F)[CH0[h] : CH0[h] + chh]
            onehot = (cls_h[..., None] == np.arange(1, 4)).astype(np.uint8)
            # [ch_h, p, f, c] -> [p, f, ch_h, c]
            msk[:, CH0[h] * F * 3 : (CH0[h] + chh) * F * 3] = (
                onehot.transpose(1, 2, 0, 3).reshape(SUB, chh * F * 3))

        cont = np.ascontiguousarray(
            continuous[rows].reshape(CH, SUB, NCONT)
            .transpose(1, 0, 2).reshape(SUB, CH * NCONT)).astype(BF)

        in_maps.append({**common, "idx": idx, "msk": msk, "cont": cont})
    return in_maps


_NC_CACHE = {}


def kernel(**inputs) -> np.ndarray:
    if "nc" not in _NC_CACHE:
        _NC_CACHE["nc"] = build_kernel()
    nc = _NC_CACHE["nc"]
    in_maps = prep_inputs(**inputs)
    res = run_bass_kernel_spmd(nc, in_maps, core_ids=list(range(NCORES)))
    out = np.concatenate(
        [r["y"].reshape(BC, 1) for r in res.results], axis=0)
    return out.astype(np.float32)


# revision 40
# speedup vs baseline: 1.1814x; 1.0165x over previous
"""DeepFM (embedding_lookup) Trainium2 Bass kernel.

Sharding: data-parallel on batch across 8 NeuronCores; the embedding
table is replicated per core in bf16, padded to 64B rows packed 4-per-
256B slot so the SWDGE dma_gather ucode (int16 slot indices, 256B-
multiple stride) can fetch each sample-feature row as one descriptor.
Gathers are spread over 4 SWDGE queues (4 Q7 cpu pairs generate
descriptors concurrently). A predicated-copy tree on the vector engine
selects the right 64B row out of each 256B slot, split by feature
group so selection overlaps the remaining gathers. FM statistics are
computed from the pre-reorder (feature-major) layout so only the
transposes wait on the X' reorder. The batch is gathered in two uneven
halves (12 chunks + 4 chunks) so most of the MLP overlaps the second
gather wave and only one block remains after the last gather. MLP runs
in bf16 with fp32 PSUM; the FM row is accumulated straight into the
output-layer PSUM via identity matmuls.

Self-contained: hardcodes all shapes from the problem spec.
"""

import numpy as np
import ml_dtypes

import concourse.bass as bass
import concourse.bacc as bacc
import concourse.mybir as mybir
import concourse.tile as tile
from concourse.bass_utils import run_bass_kernel_spmd
from concourse.masks import make_identity

F32 = mybir.dt.float32
BF16 = mybir.dt.bfloat16
I16 = mybir.dt.int16
U8 = mybir.dt.uint8
AF = mybir.ActivationFunctionType
ALU = mybir.AluOpType
BF = ml_dtypes.bfloat16

# Problem dims
B, NCONT, F, V, D = 16384, 13, 26, 100000, 16
H1, H2 = 400, 400
NCORES = 8
BC = B // NCORES          # 2048 rows per core
SUB = 128                 # batch subtile (partition dim)
CH = BC // SUB            # 16 chunks per core
NSUB = 4                  # subtiles per block
BLK = SUB * NSUB          # 512 rows per block
NBLK = BC // BLK          # 4 blocks per core
CH_SPLIT = (8, 8)         # chunks per gather half
BLK_SPLIT = ((0, 1), (2, 3))
NHALF = 2
NG = 2                    # select feature-groups per half
FG = F // NG              # features per group = 13
W17 = D + 1               # emb row: 16 emb + 1 emb_first
J18 = 18                  # select copies 18 els (18th is table pad 0)
XW = NCONT + F * W17      # 455 = X' row width
NSLOT = V // 4            # 256B-slots per feature (4 rows of 64B each)
CH0 = (0, CH_SPLIT[0])    # chunk offset of each half
IDXW = tuple(c * SUB // 16 for c in CH_SPLIT)   # idx words per (f, half)
IDX_COLS = F * sum(IDXW)


def _chunks(total, step=128):
    return [(s, min(step, total - s)) for s in range(0, total, step)]


def build_kernel():
    KCH = _chunks(XW)          # X' K-chunks: 128,128,128,71
    MCH1 = _chunks(H1)         # L1 M-tiles == L2 K-chunks
    MCH2 = _chunks(H2)         # L2 M-tiles == out-layer K-chunks
    n_wo_ch = len(MCH2)

    nc = bacc.Bacc("TRN2", target_bir_lowering=False, debug=False,
                   num_swdge_queues=4)

    t_tab = nc.dram_tensor("tab", [F * NSLOT, 128], BF16, kind="ExternalInput")
    t_idx = nc.dram_tensor("idx", [128, IDX_COLS], I16, kind="ExternalInput")
    t_msk = nc.dram_tensor("msk", [128, F * CH * 3], U8, kind="ExternalInput")
    t_cont = nc.dram_tensor("cont", [128, CH * NCONT], BF16, kind="ExternalInput")
    t_w1 = nc.dram_tensor("w1p", [128, len(KCH) * H1], BF16, kind="ExternalInput")
    t_w2 = nc.dram_tensor("w2", [128, len(MCH1) * H2], BF16, kind="ExternalInput")
    t_b1 = nc.dram_tensor("b1", [128, len(MCH1)], F32, kind="ExternalInput")
    t_b2 = nc.dram_tensor("b2", [128, len(MCH2)], F32, kind="ExternalInput")
    t_wo = nc.dram_tensor("wo", [128, n_wo_ch], BF16, kind="ExternalInput")
    t_wc = nc.dram_tensor("wc", [128, NCONT], BF16, kind="ExternalInput")
    t_fs4 = nc.dram_tensor("fs4", [128, 4], F32, kind="ExternalInput")
    t_ob = nc.dram_tensor("ob", [1, 1], F32, kind="ExternalInput")
    t_y = nc.dram_tensor("y", [NBLK, 1, BLK], F32, kind="ExternalOutput")

    with tile.TileContext(nc) as tc:
        with (
            tc.tile_pool(name="wpool", bufs=1) as wpool,
            tc.tile_pool(name="gpool", bufs=1) as gpool,
            tc.tile_pool(name="cpool", bufs=1) as cpool,
            tc.tile_pool(name="txpool", bufs=2) as txpool,
            tc.tile_pool(name="xpool", bufs=2) as xpool,
            tc.tile_pool(name="hpool", bufs=2) as hpool,
            tc.tile_pool(name="opool", bufs=2) as opool,
            tc.tile_pool(name="pt_ps", bufs=2, space="PSUM") as pt_ps,
            tc.tile_pool(name="mm1_ps", bufs=3, space="PSUM") as mm1_ps,
            tc.tile_pool(name="mm2_ps", bufs=2, space="PSUM") as mm2_ps,
            tc.tile_pool(name="o_ps", bufs=1, space="PSUM") as o_ps,
        ):
            # ---- idx on the Activation HWDGE stream so gathers are not
            # gated by the bulk weight loads on the sync stream ----
            idx_sb = wpool.tile([128, IDX_COLS], I16)
            col = 0
            idx_off = []
            for h in range(NHALF):
                idx_off.append(col)
                w = F * IDXW[h]
                nc.scalar.dma_start(
                    out=idx_sb[:, col : col + w], in_=t_idx[:, col : col + w])
                col += w
            msk_sb = wpool.tile([128, F * CH * 3], U8)
            nc.sync.dma_start(out=msk_sb[:], in_=t_msk[:])

            ident = wpool.tile([128, 128], BF16)
            make_identity(nc, ident)
            identf = wpool.tile([128, 128], F32)
            make_identity(nc, identf)

            w1all = wpool.tile([128, len(KCH) * H1], BF16)
            nc.sync.dma_start(out=w1all[:], in_=t_w1[:])
            w2all = wpool.tile([128, len(MCH1) * H2], BF16)
            nc.sync.dma_start(out=w2all[:], in_=t_w2[:])
            b1t = wpool.tile([128, len(MCH1)], F32)
            nc.sync.dma_start(out=b1t[:], in_=t_b1[:])
            b2t = wpool.tile([128, len(MCH2)], F32)
            nc.sync.dma_start(out=b2t[:], in_=t_b2[:])
            wo_sb = wpool.tile([128, n_wo_ch], BF16)
            nc.sync.dma_start(out=wo_sb[:], in_=t_wo[:])
            wc_sb = wpool.tile([128, NCONT], BF16)
            nc.sync.dma_start(out=wc_sb[:], in_=t_wc[:])
            fs4_sb = wpool.tile([128, 4], F32)
            nc.sync.dma_start(out=fs4_sb[:], in_=t_fs4[:])
            ob_sb = wpool.tile([1, 1], F32)
            nc.sync.dma_start(out=ob_sb[:], in_=t_ob[:])

            # ---- X' tile for the whole core: [p, ch, 455] bf16 ----
            xp = wpool.tile([128, CH * XW], BF16)
            xp_ap = xp[:]
            nc.sync.dma_start(
                out=bass.AP(tensor=xp.tensor, offset=xp_ap.offset,
                            ap=[xp_ap.ap[0], [XW, CH], [1, NCONT]]),
                in_=t_cont[:].rearrange("p (c w) -> p c w", w=NCONT),
            )

            nidx_reg = nc.gpsimd.to_reg(CH_SPLIT[0] * SUB)
            fmv_h = []

            def _emit_block(blk, h):
                # transpose X' -> xT chunks [128, 512] bf16
                xt_sb = []
                for ci, (k0, ks) in enumerate(KCH):
                    pt = pt_ps.tile([128, BLK], BF16, tag="pt")
                    for s in range(NSUB):
                        ch = blk * NSUB + s
                        nc.tensor.transpose(
                            out=pt[0:ks, s * SUB : (s + 1) * SUB],
                            in_=bass.AP(tensor=xp.tensor,
                                        offset=xp_ap.offset + ch * XW + k0,
                                        ap=[xp_ap.ap[0], [1, ks]]),
                            identity=ident[:],
                        )
                    xt = xpool.tile([128, BLK], BF16, tag=f"xt{ci}")
                    nc.scalar.copy(out=xt[0:ks, :], in_=pt[0:ks, :])
                    xt_sb.append(xt)

                # L1: h1^T = relu(W1'^T X'^T + b1)
                h1_sb = []
                for mi, (m0, ms) in enumerate(MCH1):
                    ps1 = mm1_ps.tile([128, BLK], F32, tag="ps1")
                    for ci, (k0, ks) in enumerate(KCH):
                        nc.tensor.matmul(
                            out=ps1[0:ms, :],
                            lhsT=w1all[0:ks, ci * H1 + m0 : ci * H1 + m0 + ms],
                            rhs=xt_sb[ci][0:ks, :],
                            start=(ci == 0), stop=(ci == len(KCH) - 1),
                        )
                    h1m = hpool.tile([128, BLK], BF16, tag=f"h1m{mi}")
                    nc.scalar.activation(
                        out=h1m[0:ms, :], in_=ps1[0:ms, :], func=AF.Relu,
                        bias=b1t[0:ms, mi : mi + 1],
                    )
                    h1_sb.append(h1m)

                # L2: h2^T = relu(W2^T h1^T + b2)
                h2_sb = []
                for mi, (m0, ms) in enumerate(MCH2):
                    ps2 = mm2_ps.tile([128, BLK], F32, tag="ps2")
                    for ci, (k0, ks) in enumerate(MCH1):
                        nc.tensor.matmul(
                            out=ps2[0:ms, :],
                            lhsT=w2all[0:ks, ci * H2 + m0 : ci * H2 + m0 + ms],
                            rhs=h1_sb[ci][0:ks, :],
                            start=(ci == 0), stop=(ci == len(MCH1) - 1),
                        )
                    h2m = hpool.tile([128, BLK], BF16, tag=f"h2m{mi}")
                    nc.scalar.activation(
                        out=h2m[0:ms, :], in_=ps2[0:ms, :], func=AF.Relu,
                        bias=b2t[0:ms, mi : mi + 1],
                    )
                    h2_sb.append(h2m)

                # out: y = W_out[1:]^T h2^T + fm + b; fm accumulated into the
                # same PSUM via identity-matmul of fmv columns
                pso = o_ps.tile([1, BLK], F32, tag="pso")
                for ci, (k0, ks) in enumerate(MCH2):
                    nc.tensor.matmul(
                        out=pso[0:1, :],
                        lhsT=wo_sb[0:ks, ci : ci + 1],
                        rhs=h2_sb[ci][0:ks, :],
                        start=(ci == 0), stop=False,
                    )
                fmv = fmv_h[h]
                for s in range(NSUB):
                    col = blk * NSUB + s - CH0[h]
                    nc.tensor.matmul(
                        out=pso[0:1, s * SUB : (s + 1) * SUB],
                        lhsT=fmv[:, col : col + 1],
                        rhs=identf[:],
                        start=False, stop=True,
                    )
                orow = opool.tile([1, BLK], F32, tag="orow")
                nc.scalar.activation(
                    out=orow[:], in_=pso[0:1, :], func=AF.Identity,
                    bias=ob_sb[0:1, :],
                )
                nc.sync.dma_start(out=t_y[blk], in_=orow[:])

            for h in range(NHALF):
                CHH = CH_SPLIT[h]
                NIDX = CHH * SUB
                KG = FG * CHH
                # ---- gather: dma_gather per feature (sub-launches of <=1024
                # idxs: the ucode misbehaves beyond 1024), 4 queues ----
                g = gpool.tile([128, F * NIDX], BF16, tag=f"G{h}")
                qn = 0
                for f in range(F):
                    off = 0
                    while off < NIDX:
                        n = min(1024, NIDX - off)
                        nc.gpsimd.dma_gather(
                            out_ap=g[
                                :, f * NIDX + off : f * NIDX + off + n
                            ].rearrange("p (c u) -> p c u", u=128),
                            in_ap=t_tab[f * NSLOT : (f + 1) * NSLOT, :],
                            idxs_ap=idx_sb[
                                :, idx_off[h] + f * IDXW[h] + off // 16
                                   : idx_off[h] + f * IDXW[h] + (off + n) // 16
                            ],
                            num_idxs=n,
                            num_idxs_reg=nidx_reg,
                            elem_size=128,
                            single_packet=False,
                            queue_num=qn % 4,
                        )
                        qn += 1
                        off += n

                g_ap = g[:]
                m_base = msk_sb[:].offset + CH0[h] * F * 3
                tx_grp = []
                for grp in range(NG):
                    # class select for features [grp*FG, (grp+1)*FG):
                    # tx[p, kg, 0:18] = g[p, grp*KG*128 + 128*kg + 32c + j]
                    def g_slice(c):
                        return bass.AP(
                            tensor=g.tensor,
                            offset=g_ap.offset + grp * KG * 128 + 32 * c,
                            ap=[g_ap.ap[0], [128, KG], [1, J18]])

                    m_off = m_base + grp * KG * 3

                    def m_slice(ci):
                        return bass.AP(
                            tensor=msk_sb.tensor, offset=m_off + ci,
                            ap=[msk_sb[:].ap[0], [3, KG], [0, J18]])

                    tx = txpool.tile([128, KG * J18], BF16, tag=f"ctx{h}")
                    tx3 = tx[:].rearrange("p (k j) -> p k j", j=J18)
                    nc.vector.tensor_copy(tx3, g_slice(0))
                    for c in (1, 2, 3):
                        nc.vector.copy_predicated(
                            out=tx3, mask=m_slice(c - 1), data=g_slice(c))
                    tx_grp.append(tx)

                    # reorder (f-major in group) -> X'[p, ch, 13+17f+j]
                    tx_ap = tx[:]
                    src = bass.AP(tensor=tx.tensor, offset=tx_ap.offset,
                                  ap=[tx_ap.ap[0], [J18, CHH],
                                      [CHH * J18, FG], [1, W17]])
                    dst = bass.AP(
                        tensor=xp.tensor,
                        offset=(xp_ap.offset + CH0[h] * XW + NCONT
                                + grp * FG * W17),
                        ap=[xp_ap.ap[0], [XW, CHH], [W17, FG], [1, W17]])
                    nc.scalar.copy(out=dst, in_=src)

                # ---- FM terms from the feature-major tx tiles ----
                seg, r2g, rfg = [], [], []
                for grp in range(NG):
                    tx_ap = tx_grp[grp][:]
                    txt = tx_grp[grp].tensor
                    se_p = cpool.tile([128, CHH * D], F32, name=f"se{grp}h{h}")
                    nc.vector.tensor_reduce(
                        out=se_p[:].rearrange("p (c d) -> p c d", d=D),
                        in_=bass.AP(tensor=txt, offset=tx_ap.offset,
                                    ap=[tx_ap.ap[0], [J18, CHH], [1, D],
                                        [CHH * J18, FG]]),
                        axis=mybir.AxisListType.X, op=ALU.add,
                    )
                    seg.append(se_p)
                    sq = cpool.tile([128, KG * J18], BF16, name=f"sq{grp}h{h}")
                    nc.vector.tensor_mul(out=sq[:], in0=tx_ap, in1=tx_ap)
                    r2_p = cpool.tile([128, CHH], F32, name=f"r2{grp}h{h}")
                    nc.vector.tensor_reduce(
                        out=r2_p[:],
                        in_=bass.AP(tensor=sq.tensor, offset=sq[:].offset,
                                    ap=[sq[:].ap[0], [J18, CHH],
                                        [CHH * J18, FG], [1, D]]),
                        axis=mybir.AxisListType.XY, op=ALU.add,
                    )
                    r2g.append(r2_p)
                    rf_p = cpool.tile([128, CHH], F32, name=f"rf{grp}h{h}")
                    nc.vector.tensor_reduce(
                        out=rf_p[:],
                        in_=bass.AP(tensor=txt, offset=tx_ap.offset + D,
                                    ap=[tx_ap.ap[0], [J18, CHH],
                                        [CHH * J18, FG]]),
                        axis=mybir.AxisListType.X, op=ALU.add,
                    )
                    rfg.append(rf_p)

                se = cpool.tile([128, CHH * D], F32, name=f"seh{h}")
                nc.vector.tensor_add(out=se[:], in0=seg[0][:], in1=seg[1][:])
                se2 = cpool.tile([128, CHH * D], F32, name=f"se2h{h}")
                nc.vector.tensor_mul(out=se2[:], in0=se[:], in1=se[:])
                rr = cpool.tile([128, CHH * 4], F32, name=f"rrh{h}")
                rr_ap = rr[:]

                def rr_slice(idx):
                    return bass.AP(tensor=rr.tensor, offset=rr_ap.offset + idx,
                                   ap=[rr_ap.ap[0], [4, CHH]])

                nc.vector.tensor_reduce(
                    out=rr_slice(0),
                    in_=se2[:].rearrange("p (c d) -> p c d", d=D),
                    axis=mybir.AxisListType.X, op=ALU.add,
                )
                nc.vector.tensor_add(out=rr_slice(1), in0=r2g[0][:],
                                     in1=r2g[1][:])
                nc.vector.tensor_add(out=rr_slice(2), in0=rfg[0][:],
                                     in1=rfg[1][:])
                # r3 = cont . W_cont
                cw = cpool.tile([128, CHH * NCONT], F32, name=f"cwh{h}")
                nc.vector.tensor_mul(
                    out=cw[:].rearrange("p (c w) -> p c w", w=NCONT),
                    in0=bass.AP(tensor=xp.tensor,
                                offset=xp_ap.offset + CH0[h] * XW,
                                ap=[xp_ap.ap[0], [XW, CHH], [1, NCONT]]),
                    in1=bass.AP(tensor=wc_sb.tensor, offset=wc_sb[:].offset,
                                ap=[wc_sb[:].ap[0], [0, CHH], [1, NCONT]]),
                )
                nc.vector.tensor_reduce(
                    out=rr_slice(3),
                    in_=cw[:].rearrange("p (c w) -> p c w", w=NCONT),
                    axis=mybir.AxisListType.X, op=ALU.add,
                )
                # fmv = w_fm * (0.5 r1 - 0.5 r2 + rf + r3)
                ft = cpool.tile([128, CHH * 4], F32, name=f"fth{h}")
                nc.vector.tensor_mul(
                    out=ft[:].rearrange("p (c k) -> p c k", k=4),
                    in0=rr[:].rearrange("p (c k) -> p c k", k=4),
                    in1=bass.AP(tensor=fs4_sb.tensor, offset=fs4_sb[:].offset,
                                ap=[fs4_sb[:].ap[0], [0, CHH], [1, 4]]),
                )
                fmv = cpool.tile([128, CHH], F32, name=f"fmvh{h}")
                nc.vector.tensor_reduce(
                    out=fmv[:],
                    in_=ft[:].rearrange("p (c k) -> p c k", k=4),
                    axis=mybir.AxisListType.X, op=ALU.add,
                )
                fmv_h.append(fmv)

                # ---- MLP for this half's blocks ----
                for blk in BLK_SPLIT[h]:
                    _emit_block(blk, h)

    nc.compile()
    return nc


def prep_inputs(continuous, cat_idx, W_cont, b_cont, emb_first, emb, W1, b1,
                W2, b2, W_out, b_out):
    """Host-side: padded bf16 table, wrapped int16 slot indices, class masks,
    bf16 weights, per-core shards."""
    KCH = _chunks(XW)
    MCH1 = _chunks(H1)
    MCH2 = _chunks(H2)

    # padded table: 4 rows of 64B per 256B slot; row = 16 emb + first + pad
    emb = np.asarray(emb, np.float32)                     # [F, V, D]
    emb_first = np.asarray(emb_first, np.float32)         # [F, V]
    tab = np.zeros((F, NSLOT, 4, 32), BF)
    tab[..., :D] = emb.reshape(F, NSLOT, 4, D).astype(BF)
    tab[..., D] = emb_first.reshape(F, NSLOT, 4).astype(BF)
    tab = np.ascontiguousarray(tab.reshape(F * NSLOT, 128))

    r_all = np.asarray(cat_idx).astype(np.int64)          # [B, F]

    W1 = np.asarray(W1, np.float32)
    w1p = np.zeros((XW, H1), np.float32)
    w1p[0:NCONT] = W1[0:NCONT]
    for f in range(F):
        w1p[NCONT + W17 * f : NCONT + W17 * f + D] = (
            W1[NCONT + D * f : NCONT + D * f + D])
    w1pk = np.zeros((128, len(KCH) * H1), BF)
    for ci, (k0, ks) in enumerate(KCH):
        w1pk[0:ks, ci * H1 : (ci + 1) * H1] = w1p[k0 : k0 + ks].astype(BF)

    W2 = np.asarray(W2, np.float32)
    w2k = np.zeros((128, len(MCH1) * H2), BF)
    for ci, (k0, ks) in enumerate(MCH1):
        w2k[0:ks, ci * H2 : (ci + 1) * H2] = W2[k0 : k0 + ks].astype(BF)

    b1 = np.asarray(b1, np.float32)
    b1t = np.zeros((128, len(MCH1)), np.float32)
    for mi, (m0, ms) in enumerate(MCH1):
        b1t[0:ms, mi] = b1[m0 : m0 + ms]
    b2 = np.asarray(b2, np.float32)
    b2t = np.zeros((128, len(MCH2)), np.float32)
    for mi, (m0, ms) in enumerate(MCH2):
        b2t[0:ms, mi] = b2[m0 : m0 + ms]

    W_out = np.asarray(W_out, np.float32)
    n_wo_ch = (H2 + 127) // 128
    wo_t = np.zeros((n_wo_ch, 128), np.float32)
    wo_t.reshape(-1)[:H2] = W_out[1:, 0]
    wo = np.ascontiguousarray(wo_t.T).astype(BF)

    w_fm = np.float32(W_out[0, 0])
    ob = np.float32(b_out[0] + w_fm * b_cont[0])
    fs4 = np.tile(
        np.array([0.5 * w_fm, -0.5 * w_fm, w_fm, w_fm], np.float32), (128, 1))

    common = {
        "tab": tab,
        "w1p": w1pk,
        "w2": w2k,
        "b1": b1t,
        "b2": b2t,
        "wo": wo,
        "wc": np.tile(np.asarray(W_cont, np.float32).reshape(1, NCONT),
                      (128, 1)).astype(BF),
        "fs4": fs4,
        "ob": np.array([[ob]], np.float32),
    }

    continuous = np.asarray(continuous, np.float32)
    in_maps = []
    for c in range(NCORES):
        rows = slice(c * BC, (c + 1) * BC)
        r = r_all[rows]                                   # [2048, F]
        q = (r >> 2).astype(np.int16)
        cls = (r & 3).astype(np.int64)

        # idx: per (half, f): flat gather i -> sample CH0[h]*128 + i,
        # wrapped i -> [i%16, i//16], replicated to 128 partitions
        idx = np.zeros((16, IDX_COLS), np.int16)
        col = 0
        for h in range(NHALF):
            n = CH_SPLIT[h] * SUB
            qs = q[CH0[h] * SUB : CH0[h] * SUB + n]       # [n, F]
            w = qs.reshape(n // 16, 16, F).transpose(1, 2, 0).reshape(
                16, F * (n // 16))
            idx[:, col : col + w.shape[1]] = w
            col += w.shape[1]
        idx = np.tile(idx, (8, 1))

        # class masks for copy_predicated, laid out per half:
        # col = CH0[h]*F*3 + (f*CHH + ch_h)*3 + (c-1)
        msk = np.zeros((SUB, F * CH * 3), np.uint8)
        for h in range(NHALF):
            chh = CH_SPLIT[h]
            cls_h = cls.reshape(CH, SUB, F)[CH0[h] : CH0[h] + chh]
            onehot = (cls_h[..., None] == np.arange(1, 4)).astype(np.uint8)
            # [ch_h, p, f, c] -> [p, f, ch_h, c]
            msk[:, CH0[h] * F * 3 : (CH0[h] + chh) * F * 3] = (
                onehot.transpose(1, 2, 0, 3).reshape(SUB, chh * F * 3))

        cont = np.ascontiguousarray(
            continuous[rows].reshape(CH, SUB, NCONT)
            .transpose(1, 0, 2).reshape(SUB, CH * NCONT)).astype(BF)

        in_maps.append({**common, "idx": idx, "msk": msk, "cont": cont})
    return in_maps


_NC_CACHE = {}


def kernel(**inputs) -> np.ndarray:
    if "nc" not in _NC_CACHE:
        _NC_CACHE["nc"] = build_kernel()
    nc = _NC_CACHE["nc"]
    in_maps = prep_inputs(**inputs)
    res = run_bass_kernel_spmd(nc, in_maps, core_ids=list(range(NCORES)))
    out = np.concatenate(
        [r["y"].reshape(BC, 1) for r in res.results], axis=0)
    return out.astype(np.float32)
